# revision 22
# baseline (speedup 1.0000x reference)
"""DigitCaps dynamic-routing kernel for 8 TRN2 NeuronCores.

Math refactor (u_hat is NEVER materialized - it would be 189 MB):
  u_hat[b,r,c,d] = sum_i W[r,c,d,i] * u[b,r,i]
  softmax over r without max-subtraction (b_ij values are O(1)):
      c_ij[r,c,d] = exp(b[r,c,d]) / Z[c,d],  Z = sum_r exp(b)
  s[b,c,d]  = (sum_{r,i} (exp(b) * W)[r,c,d,i] u[b,r,i]) / Z[c,d]
  v = squash(s)
  b[r,c,d] += (1/B) sum_b t[b,r,c] v[b,c,d],  t[b,r,c] = sum_i Wd[r,c,i] u[b,r,i]
  The b update is computed as  sum_i (Wd[r,c,i]/B) * M[(r,i),(c,d)]  with
  M = sum_b u[b,(r,i)] v[b,(c,d)]  (outer-product matmuls, batch on
  partitions), then an 8->1 partition reduce over i via a block-diagonal
  ones matmul.  This avoids the per-(group,caps) micro-matmuls.

Sharding: routes (R=1152) split across 8 cores (144 each). Per iteration one
bf16 AllReduce carries the partial s' (B x C*D); the softmax denominator Z
travels in its own tiny (1,160) bf16 AllReduce that overlaps the s matmuls.
A dummy warmup AllReduce is issued first to absorb the one-time collective
rendezvous cost concurrently with the input DMA. v is computed redundantly
on every core; core 0's output is returned.

Per-core layout: contraction dim K = (r_local, i) = 1152 = 9 chunks of 128.
b lives as (16 rp-partitions) x (k, c, d) so exp/E-expansion batch cleanly.
Host pre-permutes u (both K-major and batch-major) and W (pure permutation -
no reference compute happens on host).
"""

import os
import numpy as np

B, R, C, D, I = 256, 1152, 10, 16, 8
CD = C * D                 # 160
NCORES = 8
RL = R // NCORES           # 144 routes per core
NCHUNK = RL * I // 128     # 9 K-chunks of 128
NG = 3                     # chunk groups of 3
NITER = 3
EPS = 1e-5

# host blob column offsets (fp32, 128 partitions)
O_UT = 0                      # uT  (128, 9*256): [(rp,i), (k, b)]
O_UB = O_UT + NCHUNK * B      # uB  (128, 2*1152): [b%128, (bh, k, rp, i)]
O_WT = O_UB + 2 * RL * I      # Wt  (128, 9*160): [(rp,i), (k, c, d)]
O_OBD = O_WT + NCHUNK * CD    # onesbd (128, 16): [(rp,i), rp']
O_EX2 = O_OBD + 16            # ex2 rows0:16 (16, 128): [rp', (rp,i)]
DW = O_EX2 + 128

_CACHE = {}


def _build_program():
    from contextlib import ExitStack

    import concourse.bass as bass
    import concourse.bacc as bacc
    import concourse.mybir as mybir
    import concourse.tile as tile

    f32 = mybir.dt.float32
    bf16 = mybir.dt.bfloat16
    AF = mybir.ActivationFunctionType
    ALU = mybir.AluOpType

    nc = bacc.Bacc(None, num_devices=NCORES)

    data_d = nc.declare_dram_parameter("data", [128, DW], f32, isOutput=False)
    out_d = nc.declare_dram_parameter("out", [B, CD], f32, isOutput=True)

    rgroups = [list(range(NCORES))]

    with tile.TileContext(nc) as tc, ExitStack() as ctx:
        singles = ctx.enter_context(tc.tile_pool(name="singles", bufs=1))
        rhpool = ctx.enter_context(tc.tile_pool(name="rh", bufs=3))
        gpool = ctx.enter_context(tc.tile_pool(name="g", bufs=3))
        stpool = ctx.enter_context(tc.tile_pool(name="stage", bufs=2))
        work = ctx.enter_context(tc.tile_pool(name="work", bufs=8))
        ps_s = ctx.enter_context(tc.tile_pool(name="ps_s", bufs=1, space="PSUM"))
        ps_m = ctx.enter_context(tc.tile_pool(name="ps_m", bufs=2, space="PSUM"))
        ps_bu = ctx.enter_context(tc.tile_pool(name="ps_bu", bufs=1, space="PSUM"))
        ps_pe = ctx.enter_context(tc.tile_pool(name="ps_pe", bufs=2, space="PSUM"))
        ps_z = ctx.enter_context(tc.tile_pool(name="ps_z", bufs=1, space="PSUM"))
        dram = ctx.enter_context(tc.tile_pool(name="dram", bufs=1, space="DRAM"))

        # collective buffers per iteration; iters>0 carry Z in cols 2CD:3CD
        cc = []
        for it in range(NITER):
            w = 2 * CD if it == 0 else 3 * CD
            ci = dram.tile([128, w], bf16, tag=f"ci{it}", name=f"ci{it}")
            co = dram.tile([128, w], bf16, tag=f"co{it}", name=f"co{it}")
            cc.append((ci, co, w))

        # ---- input DMA (3 pieces so casts can start early) ----
        sb_data = singles.tile([128, DW], f32, tag="data")
        nc.sync.dma_start(out=sb_data[:, O_UT:O_UB], in_=data_d[:, O_UT:O_UB])
        nc.sync.dma_start(out=sb_data[:, O_WT:DW], in_=data_d[:, O_WT:DW])
        nc.sync.dma_start(out=sb_data[:, O_UB:O_WT], in_=data_d[:, O_UB:O_WT])
        sb_uT = sb_data[:, O_UT:O_UT + NCHUNK * B]
        sb_uB = sb_data[:, O_UB:O_UB + 2 * RL * I]
        sb_Wt = sb_data[:, O_WT:O_WT + NCHUNK * CD]
        sb_obd = sb_data[:, O_OBD:O_OBD + 16]
        sb_ex2 = sb_data[0:16, O_EX2:O_EX2 + 128]

        # bf16 matmul operand copies for the s0 path, chunked so the first
        # matmuls start before the full casts finish
        sb_uTb = singles.tile([128, NCHUNK * B], bf16, tag="uTb")
        sb_Wtb = singles.tile([128, NCHUNK * CD], bf16, tag="Wtb")
        for g in range(NG):
            nc.vector.tensor_copy(
                out=sb_uTb[:, g * NG * B:(g + 1) * NG * B],
                in_=sb_uT[:, g * NG * B:(g + 1) * NG * B],
            )
            nc.vector.tensor_copy(
                out=sb_Wtb[:, g * NG * CD:(g + 1) * NG * CD],
                in_=sb_Wt[:, g * NG * CD:(g + 1) * NG * CD],
            )

        # everything below is only needed after AR0 returns; declared here,
        # emitted inside the loop (after the AR0 trigger) so the DVE work
        # lands in the collective-rendezvous window
        sb_uBb = singles.tile([128, 2 * RL * I], bf16, tag="uBb")
        sb_obdb = singles.tile([128, 16], bf16, tag="obdb")
        sb_ex2b = singles.tile([16, 128], bf16, tag="ex2b")
        sb_o16 = singles.tile([16, 128], bf16, tag="o16")
        sb_Wd = singles.tile([128, NCHUNK * C], f32, tag="Wd")
        sb_b = singles.tile([16, NCHUNK * CD], f32, tag="b")
        sb_E = singles.tile([16, NCHUNK * CD], bf16, tag="E")
        sb_vf = singles.tile([128, 2 * CD], f32, tag="vf")
        sb_vb = singles.tile([128, 2 * CD], bf16, tag="vb")

        def emit_deferred_setup():
            nc.vector.tensor_copy(out=sb_uBb, in_=sb_uB)
            nc.vector.tensor_copy(out=sb_obdb, in_=sb_obd)
            nc.vector.tensor_copy(out=sb_ex2b, in_=sb_ex2)
            nc.vector.memset(sb_o16, 1.0)
            # Wd[(rp,i), (k,c)] = (1/B) * sum_d Wt
            for k in range(NCHUNK):
                nc.vector.reduce_sum(
                    out=sb_Wd[:, k * C:(k + 1) * C],
                    in_=sb_Wt[:, k * CD:(k + 1) * CD].rearrange(
                        "p (c d) -> p c d", d=D),
                    axis=mybir.AxisListType.X,
                )
            nc.vector.tensor_scalar_mul(sb_Wd, sb_Wd, 1.0 / B)
            nc.vector.memset(sb_b, 0.0)

        for it in range(NITER):
            ci, co, w = cc[it]
            if it > 0:
                # per-group: exp -> Z-reduce, E-expand, rhs mul (pipelined)
                rhs, zgs = [], []
                for g in range(NG):
                    Eg = sb_E[:, g * NG * CD:(g + 1) * NG * CD]
                    nc.scalar.activation(
                        out=Eg, in_=sb_b[:, g * NG * CD:(g + 1) * NG * CD],
                        func=AF.Exp,
                    )
                    zg = work.tile([16, CD], f32, tag=f"zg{g}", name=f"zg{g}")
                    e_kv = bass.AP(
                        tensor=Eg.tensor, offset=Eg.offset,
                        ap=[Eg.ap[0], [1, CD], [CD, NG]],
                    )
                    nc.vector.reduce_sum(out=zg, in_=e_kv, axis=mybir.AxisListType.X)
                    zgs.append(zg)
                    peg = ps_pe.tile([128, NG * CD], f32, tag="pe")
                    nc.tensor.matmul(peg, sb_ex2b, Eg, start=True, stop=True)
                    rg = rhpool.tile([128, NG * CD], bf16, tag="rh")
                    nc.vector.tensor_mul(
                        rg, sb_Wt[:, g * NG * CD:(g + 1) * NG * CD], peg
                    )
                    rhs.append(rg)
                zp16f = work.tile([16, CD], f32, tag="zp16f")
                nc.vector.tensor_add(zp16f, zgs[0], zgs[1])
                nc.vector.tensor_add(zp16f, zp16f, zgs[2])
                zp16 = work.tile([16, CD], bf16, tag="zp16")
                nc.vector.tensor_copy(out=zp16, in_=zp16f)
                zrep = ps_z.tile([128, CD], f32, tag="zp")
                nc.tensor.matmul(zrep, sb_o16, zp16, start=True, stop=True)

            # s'[bh][b, cd] accumulated over 9 K-chunks; bh-major so the h0
            # stage-cast overlaps the h1 matmuls
            st = [ps_s.tile([128, CD], f32, tag=f"s{bh}", name=f"s{bh}") for bh in range(2)]
            stage = stpool.tile([128, w], bf16, tag="stage")
            for bh in range(2):
                for k in range(NCHUNK):
                    if it > 0:
                        rk = rhs[k // NG][:, (k % NG) * CD:(k % NG + 1) * CD]
                    else:
                        rk = sb_Wtb[:, k * CD:(k + 1) * CD]
                    nc.tensor.matmul(
                        st[bh],
                        sb_uTb[:, k * B + bh * 128: k * B + (bh + 1) * 128],
                        rk,
                        start=(k == 0), stop=(k == NCHUNK - 1),
                    )
                nc.vector.tensor_copy(out=stage[:, bh * CD:(bh + 1) * CD], in_=st[bh])
            if it > 0:
                nc.scalar.copy(out=stage[:, 2 * CD:3 * CD], in_=zrep)
            nc.sync.dma_start(out=ci[:], in_=stage)
            nc.gpsimd.collective_compute(
                "AllReduce", mybir.AluOpType.add,
                replica_groups=rgroups, ins=[ci.opt()], outs=[co.opt()],
            )
            if it == 0:
                emit_deferred_setup()
            red = stpool.tile([128, w], bf16, tag="red")
            if it > 0:
                nc.sync.dma_start(out=red[:, 2 * CD:3 * CD], in_=co[:, 2 * CD:3 * CD])
            nc.sync.dma_start(out=red[:, 0:2 * CD], in_=co[:, 0:2 * CD])

            # v = squash(y/z) with y = s_sum, z = Z (it0: z == R), computed
            # without dividing:  v = y*y^2 / ((z^2+y^2)(|y| + eps*z))
            # final iteration runs per batch-half so the output DMA starts early
            halves = [(0, 2 * CD)] if it < NITER - 1 else [(0, CD), (CD, 2 * CD)]
            if it > 0:
                zsq = work.tile([128, CD], f32, tag="zsq")
                nc.vector.tensor_mul(zsq, red[:, 2 * CD:3 * CD],
                                     red[:, 2 * CD:3 * CD])
            for lo, hi in halves:
                hw_ = hi - lo
                nh = hw_ // CD
                y = red[:, lo:hi]
                sq = work.tile([128, hw_], f32, tag=f"sq{lo}", name=f"sq{lo}")
                nc.vector.tensor_mul(sq, y, y)
                ay = work.tile([128, hw_], bf16, tag=f"ax{lo}", name=f"ax{lo}")
                nc.vector.tensor_scalar(
                    ay.bitcast(mybir.dt.uint16), y.bitcast(mybir.dt.uint16),
                    0x7FFF, None, ALU.bitwise_and,
                )
                d2 = work.tile([128, hw_], f32, tag=f"d2{lo}", name=f"d2{lo}")
                den = work.tile([128, hw_], f32, tag=f"dn{lo}", name=f"dn{lo}")
                if it == 0:
                    nc.vector.tensor_scalar_add(d2, sq, float(R) * R)
                    aze = work.tile([128, hw_], f32, tag=f"az{lo}", name=f"az{lo}")
                    nc.vector.tensor_scalar_add(aze, ay, EPS * R)
                    nc.vector.tensor_mul(den, d2, aze)
                else:
                    z2 = bass.AP(tensor=zsq.tensor, offset=zsq.offset,
                                 ap=[zsq.ap[0], [0, nh], [1, CD]])
                    nc.vector.tensor_add(
                        d2.rearrange("p (h f) -> p h f", f=CD),
                        sq.rearrange("p (h f) -> p h f", f=CD),
                        z2,
                    )
                    rz_sl = red[:, 2 * CD:3 * CD]
                    zb = bass.AP(tensor=rz_sl.tensor, offset=rz_sl.offset,
                                 ap=[rz_sl.ap[0], [0, nh], [1, CD]])
                    aze = work.tile([128, hw_], f32, tag=f"az{lo}", name=f"az{lo}")
                    nc.vector.scalar_tensor_tensor(
                        out=aze.rearrange("p (h f) -> p h f", f=CD),
                        in0=zb, scalar=EPS,
                        in1=ay.rearrange("p (h f) -> p h f", f=CD),
                        op0=ALU.mult, op1=ALU.add,
                    )
                    nc.vector.tensor_mul(den, d2, aze)
                rr = work.tile([128, hw_], f32, tag=f"rr{lo}", name=f"rr{lo}")
                nc.vector.reciprocal_approx_fast(out=rr, in_=den)
                n1 = work.tile([128, hw_], f32, tag=f"n1{lo}", name=f"n1{lo}")
                nc.vector.tensor_mul(n1, y, sq)
                if it < NITER - 1:
                    # v feeds only the bf16 b-update matmuls
                    nc.vector.tensor_mul(sb_vb, n1, rr)
                else:
                    nc.vector.tensor_mul(sb_vf[:, lo:hi], n1, rr)
                    bh = lo // CD
                    nc.sync.dma_start(
                        out=out_d[bh * 128:(bh + 1) * 128, :],
                        in_=sb_vf[:, lo:hi],
                    )

            if it < NITER - 1:
                # b += sum_i (Wd/B) * (u x v):  M_k = sum_b u v  (batch on parts)
                for g in range(NG):
                    bu = ps_bu.tile([16, NG * CD], f32, tag="bu")
                    gg = gpool.tile([128, NG * CD], bf16, tag="gk")
                    for j in range(NG):
                        k = g * NG + j
                        mp = ps_m.tile([128, CD], f32, tag="mp")
                        for bh in range(2):
                            nc.tensor.matmul(
                                mp,
                                sb_uBb[:, bh * RL * I + k * 128: bh * RL * I + (k + 1) * 128],
                                sb_vb[:, bh * CD:(bh + 1) * CD],
                                start=(bh == 0), stop=(bh == 1),
                            )
                        wd_b = bass.AP(
                            tensor=sb_Wd.tensor, offset=sb_Wd.offset + k * C,
                            ap=[sb_Wd.ap[0], [1, C], [0, D]],
                        )
                        nc.vector.tensor_mul(
                            gg[:, j * CD:(j + 1) * CD].rearrange(
                                "p (c d) -> p c d", d=D),
                            mp.rearrange("p (c d) -> p c d", d=D),
                            wd_b,
                        )
                    # one i-reduce matmul per group (n=480)
                    nc.tensor.matmul(bu, sb_obdb, gg, start=True, stop=True)
                    nc.vector.tensor_add(
                        sb_b[:, g * NG * CD:(g + 1) * NG * CD],
                        sb_b[:, g * NG * CD:(g + 1) * NG * CD],
                        bu,
                    )

    nc.compile()
    return nc


def _host_inputs(u, W):
    """Pure-permutation host prep: per-core (r,i)-major and b-major layouts."""
    u = np.ascontiguousarray(u, dtype=np.float32)
    W = np.ascontiguousarray(W, dtype=np.float32)
    obd = np.zeros((128, 16), dtype=np.float32)
    for p in range(128):
        obd[p, p // 8] = 1.0
    ex2 = obd.T.copy()  # (16, 128)
    in_maps = []
    for cidx in range(NCORES):
        rs = cidx * RL
        usl = u[:, rs:rs + RL, :].reshape(B, RL * I)           # (256, 1152)
        uTd = (usl.T.reshape(NCHUNK, 128, B)
               .transpose(1, 0, 2).reshape(128, NCHUNK * B))
        uBd = (usl.reshape(2, 128, RL * I)
               .transpose(1, 0, 2).reshape(128, 2 * RL * I))
        wsl = W[rs:rs + RL].transpose(0, 3, 1, 2).reshape(RL * I, CD)
        Wtd = (wsl.reshape(NCHUNK, 128, CD)
               .transpose(1, 0, 2).reshape(128, NCHUNK * CD))
        data = np.zeros((128, DW), dtype=np.float32)
        data[:, O_UT:O_UT + NCHUNK * B] = uTd
        data[:, O_UB:O_UB + 2 * RL * I] = uBd
        data[:, O_WT:O_WT + NCHUNK * CD] = Wtd
        data[:, O_OBD:O_OBD + 16] = obd
        data[:16, O_EX2:O_EX2 + 128] = ex2
        in_maps.append({"data": data})
    return in_maps


def _install_profile_hook():
    """Recreate the missing antenv.axon_hooks NTFF-profile hook (dev only)."""
    import contextlib
    import ctypes
    import sys
    import types

    try:
        from antenv.axon_hooks import get_axon_ntff_profile_hook  # noqa: F401
        return
    except ImportError:
        pass

    mod = types.ModuleType("antenv.axon_hooks")
    holder = {}
    mod.set_axon_ntff_profile_hook = lambda h: holder.__setitem__("h", h)
    mod.get_axon_ntff_profile_hook = lambda: holder.get("h")
    import antenv

    sys.modules["antenv.axon_hooks"] = mod
    antenv.axon_hooks = mod

    so_path = "/opt/axon/libaxon_pjrt.so"
    lib = ctypes.CDLL(so_path)
    if not hasattr(lib, "axon_start_nrt_profile"):
        return
    lib.axon_start_nrt_profile.argtypes = [
        ctypes.POINTER(ctypes.c_int64),
        ctypes.c_size_t,
    ]
    lib.axon_start_nrt_profile.restype = ctypes.c_int64
    lib.axon_stop_nrt_profile.argtypes = [ctypes.c_char_p]
    lib.axon_stop_nrt_profile.restype = ctypes.c_int64

    @contextlib.contextmanager
    def _hook(output_dir, device_ids):
        import jax

        jax.devices()
        if device_ids:
            ids = (ctypes.c_int64 * len(device_ids))(*device_ids)
            rc = lib.axon_start_nrt_profile(ids, len(device_ids))
        else:
            rc = lib.axon_start_nrt_profile(None, 0)
        if rc != 0:
            raise RuntimeError(f"axon_start_nrt_profile rc={rc}")
        try:
            yield
        finally:
            n = lib.axon_stop_nrt_profile(str(output_dir).encode())
            print(f"profile: {n} file(s) written to {output_dir}")

    mod.set_axon_ntff_profile_hook(_hook)

    # Avoid the bucket upload inside the trace post-processing.
    import concourse.bass_utils as bu

    bu.upload_artifacts = lambda tmpdir: f"local:{tmpdir}"


def kernel(u, W):
    from concourse.bass_utils import run_bass_kernel_spmd

    if os.environ.get("KERNEL_TRACE", "0") == "1":
        _install_profile_hook()
    if "nc" not in _CACHE:
        _CACHE["nc"] = _build_program()
    nc = _CACHE["nc"]
    in_maps = _host_inputs(u, W)
    trace = os.environ.get("KERNEL_TRACE", "0") == "1"
    res = run_bass_kernel_spmd(
        nc, in_maps, core_ids=list(range(NCORES)), trace=trace
    )
    _CACHE["last_result"] = res
    return np.asarray(res.results[0]["out"]).reshape(B, C, D)


# revision 27
# speedup vs baseline: 1.0430x; 1.0430x over previous
"""DigitCaps dynamic-routing kernel for 8 TRN2 NeuronCores.

Math refactor (u_hat is NEVER materialized - it would be 189 MB):
  u_hat[b,r,c,d] = sum_i W[r,c,d,i] * u[b,r,i]
  softmax over r without max-subtraction (b_ij values are O(1)):
      c_ij[r,c,d] = exp(b[r,c,d]) / Z[c,d],  Z = sum_r exp(b)
  s[b,c,d]  = (sum_{r,i} (exp(b) * W)[r,c,d,i] u[b,r,i]) / Z[c,d]
  v = squash(s)
  b[r,c,d] += (1/B) sum_b t[b,r,c] v[b,c,d],  t[b,r,c] = sum_i Wd[r,c,i] u[b,r,i]
  The b update is computed as  sum_i (Wd[r,c,i]/B) * M[(r,i),(c,d)]  with
  M = sum_b u[b,(r,i)] v[b,(c,d)]  (outer-product matmuls, batch on
  partitions), then an 8->1 partition reduce over i via a block-diagonal
  ones matmul.  This avoids the per-(group,caps) micro-matmuls.

Sharding: routes (R=1152) split across 8 cores (144 each). Per iteration one
bf16 AllReduce carries the partial s' (B x C*D); the softmax denominator Z
travels in its own tiny (1,160) bf16 AllReduce that overlaps the s matmuls.
A dummy warmup AllReduce is issued first to absorb the one-time collective
rendezvous cost concurrently with the input DMA. v is computed redundantly
on every core; core 0's output is returned.

Per-core layout: contraction dim K = (r_local, i) = 1152 = 9 chunks of 128.
b lives as (16 rp-partitions) x (k, c, d) so exp/E-expansion batch cleanly.
Host pre-permutes u (both K-major and batch-major) and W (pure permutation -
no reference compute happens on host).
"""

import os
import numpy as np

B, R, C, D, I = 256, 1152, 10, 16, 8
CD = C * D                 # 160
NCORES = 8
RL = R // NCORES           # 144 routes per core
NCHUNK = RL * I // 128     # 9 K-chunks of 128
NG = 3                     # chunk groups of 3
NITER = 3
EPS = 1e-5

# host blob column offsets (fp32, 128 partitions)
O_UT = 0                      # uT  (128, 9*256): [(rp,i), (k, b)]
O_UB = O_UT + NCHUNK * B      # uB  (128, 2*1152): [b%128, (bh, k, rp, i)]
O_WT = O_UB + 2 * RL * I      # Wt  (128, 9*160): [(rp,i), (k, c, d)]
O_OBD = O_WT + NCHUNK * CD    # onesbd (128, 16): [(rp,i), rp']
O_EX2 = O_OBD + 16            # ex2 rows0:16 (16, 128): [rp', (rp,i)]
DW = O_EX2 + 128

_CACHE = {}


def _build_program():
    from contextlib import ExitStack

    import concourse.bass as bass
    import concourse.bacc as bacc
    import concourse.mybir as mybir
    import concourse.tile as tile

    f32 = mybir.dt.float32
    bf16 = mybir.dt.bfloat16
    AF = mybir.ActivationFunctionType
    ALU = mybir.AluOpType

    nc = bacc.Bacc(None, num_devices=NCORES)

    data_d = nc.declare_dram_parameter("data", [128, DW], f32, isOutput=False)
    out_d = nc.declare_dram_parameter("out", [B, CD], f32, isOutput=True)

    rgroups = [list(range(NCORES))]

    with tile.TileContext(nc) as tc, ExitStack() as ctx:
        singles = ctx.enter_context(tc.tile_pool(name="singles", bufs=1))
        rhpool = ctx.enter_context(tc.tile_pool(name="rh", bufs=3))
        gpool = ctx.enter_context(tc.tile_pool(name="g", bufs=3))
        stpool = ctx.enter_context(tc.tile_pool(name="stage", bufs=2))
        work = ctx.enter_context(tc.tile_pool(name="work", bufs=8))
        ps_s = ctx.enter_context(tc.tile_pool(name="ps_s", bufs=1, space="PSUM"))
        ps_m = ctx.enter_context(tc.tile_pool(name="ps_m", bufs=2, space="PSUM"))
        ps_bu = ctx.enter_context(tc.tile_pool(name="ps_bu", bufs=1, space="PSUM"))
        ps_pe = ctx.enter_context(tc.tile_pool(name="ps_pe", bufs=2, space="PSUM"))
        ps_z = ctx.enter_context(tc.tile_pool(name="ps_z", bufs=1, space="PSUM"))
        dram = ctx.enter_context(tc.tile_pool(name="dram", bufs=1, space="DRAM"))

        # collective buffers per iteration; iters>0 carry Z in cols 2CD:3CD
        cc = []
        for it in range(NITER):
            w = 2 * CD if it == 0 else 3 * CD
            ci = dram.tile([128, w], bf16, tag=f"ci{it}", name=f"ci{it}")
            co = dram.tile([128, w], bf16, tag=f"co{it}", name=f"co{it}")
            cc.append((ci, co, w))

        # ---- input DMA (3 pieces so casts can start early) ----
        sb_data = singles.tile([128, DW], f32, tag="data")
        nc.sync.dma_start(out=sb_data[:, O_UT:O_UB], in_=data_d[:, O_UT:O_UB])
        nc.sync.dma_start(out=sb_data[:, O_WT:DW], in_=data_d[:, O_WT:DW])
        nc.sync.dma_start(out=sb_data[:, O_UB:O_WT], in_=data_d[:, O_UB:O_WT])
        sb_uT = sb_data[:, O_UT:O_UT + NCHUNK * B]
        sb_uB = sb_data[:, O_UB:O_UB + 2 * RL * I]
        sb_Wt = sb_data[:, O_WT:O_WT + NCHUNK * CD]
        sb_obd = sb_data[:, O_OBD:O_OBD + 16]
        sb_ex2 = sb_data[0:16, O_EX2:O_EX2 + 128]

        # bf16 matmul operand copies for the s0 path, chunked so the first
        # matmuls start before the full casts finish
        sb_uTb = singles.tile([128, NCHUNK * B], bf16, tag="uTb")
        sb_Wtb = singles.tile([128, NCHUNK * CD], bf16, tag="Wtb")
        for g in range(NG):
            nc.vector.tensor_copy(
                out=sb_uTb[:, g * NG * B:(g + 1) * NG * B],
                in_=sb_uT[:, g * NG * B:(g + 1) * NG * B],
            )
            nc.vector.tensor_copy(
                out=sb_Wtb[:, g * NG * CD:(g + 1) * NG * CD],
                in_=sb_Wt[:, g * NG * CD:(g + 1) * NG * CD],
            )

        # everything below is only needed after AR0 returns; declared here,
        # emitted inside the loop (after the AR0 trigger) so the DVE work
        # lands in the collective-rendezvous window
        sb_uBb = singles.tile([128, 2 * RL * I], bf16, tag="uBb")
        sb_obdb = singles.tile([128, 16], bf16, tag="obdb")
        sb_ex2b = singles.tile([16, 128], bf16, tag="ex2b")
        sb_o16 = singles.tile([16, 128], bf16, tag="o16")
        sb_Wd = singles.tile([128, NCHUNK * C], f32, tag="Wd")
        sb_b = singles.tile([16, NCHUNK * CD], f32, tag="b")
        sb_E = singles.tile([16, NCHUNK * CD], bf16, tag="E")
        sb_vf = singles.tile([128, 2 * CD], f32, tag="vf")
        sb_vb = singles.tile([128, 2 * CD], bf16, tag="vb")

        def emit_deferred_setup():
            nc.vector.tensor_copy(out=sb_uBb, in_=sb_uB)
            nc.vector.tensor_copy(out=sb_obdb, in_=sb_obd)
            nc.vector.tensor_copy(out=sb_ex2b, in_=sb_ex2)
            nc.vector.memset(sb_o16, 1.0)
            # Wd[(rp,i), (k,c)] = (1/B) * sum_d Wt
            for k in range(NCHUNK):
                nc.vector.reduce_sum(
                    out=sb_Wd[:, k * C:(k + 1) * C],
                    in_=sb_Wt[:, k * CD:(k + 1) * CD].rearrange(
                        "p (c d) -> p c d", d=D),
                    axis=mybir.AxisListType.X,
                )
            nc.vector.tensor_scalar_mul(sb_Wd, sb_Wd, 1.0 / B)
            nc.vector.memset(sb_b, 0.0)

        for it in range(NITER):
            ci, co, w = cc[it]
            if it > 0:
                # per-group: exp -> Z-reduce, E-expand, rhs mul (pipelined)
                rhs, zgs = [], []
                for g in range(NG):
                    Eg = sb_E[:, g * NG * CD:(g + 1) * NG * CD]
                    nc.scalar.activation(
                        out=Eg, in_=sb_b[:, g * NG * CD:(g + 1) * NG * CD],
                        func=AF.Exp,
                    )
                    zg = work.tile([16, CD], f32, tag=f"zg{g}", name=f"zg{g}")
                    e_kv = bass.AP(
                        tensor=Eg.tensor, offset=Eg.offset,
                        ap=[Eg.ap[0], [1, CD], [CD, NG]],
                    )
                    nc.vector.reduce_sum(out=zg, in_=e_kv, axis=mybir.AxisListType.X)
                    zgs.append(zg)
                    peg = ps_pe.tile([128, NG * CD], f32, tag="pe")
                    nc.tensor.matmul(peg, sb_ex2b, Eg, start=True, stop=True)
                    rg = rhpool.tile([128, NG * CD], bf16, tag="rh")
                    nc.vector.tensor_mul(
                        rg, sb_Wt[:, g * NG * CD:(g + 1) * NG * CD], peg
                    )
                    rhs.append(rg)
                zp16f = work.tile([16, CD], f32, tag="zp16f")
                nc.gpsimd.tensor_add(zp16f, zgs[0], zgs[1])
                nc.gpsimd.tensor_add(zp16f, zp16f, zgs[2])
                zp16 = work.tile([16, CD], bf16, tag="zp16")
                nc.gpsimd.tensor_copy(out=zp16, in_=zp16f)
                zrep = ps_z.tile([128, CD], f32, tag="zp")
                nc.tensor.matmul(zrep, sb_o16, zp16, start=True, stop=True)

            # s'[bh][b, cd] accumulated over 9 K-chunks; bh-major so the h0
            # stage-cast overlaps the h1 matmuls
            st = [ps_s.tile([128, CD], f32, tag=f"s{bh}", name=f"s{bh}") for bh in range(2)]
            stage = stpool.tile([128, w], bf16, tag="stage")
            for bh in range(2):
                for k in range(NCHUNK):
                    if it > 0:
                        rk = rhs[k // NG][:, (k % NG) * CD:(k % NG + 1) * CD]
                    else:
                        rk = sb_Wtb[:, k * CD:(k + 1) * CD]
                    nc.tensor.matmul(
                        st[bh],
                        sb_uTb[:, k * B + bh * 128: k * B + (bh + 1) * 128],
                        rk,
                        start=(k == 0), stop=(k == NCHUNK - 1),
                    )
                nc.vector.tensor_copy(out=stage[:, bh * CD:(bh + 1) * CD], in_=st[bh])
            if it > 0:
                nc.scalar.copy(out=stage[:, 2 * CD:3 * CD], in_=zrep)
            nc.sync.dma_start(out=ci[:], in_=stage)
            nc.gpsimd.collective_compute(
                "AllReduce", mybir.AluOpType.add,
                replica_groups=rgroups, ins=[ci.opt()], outs=[co.opt()],
            )
            if it == 0:
                emit_deferred_setup()
            red = stpool.tile([128, w], bf16, tag="red")
            nc.sync.dma_start(out=red[:, 0:2 * CD], in_=co[:, 0:2 * CD])
            if it > 0:
                nc.sync.dma_start(out=red[:, 2 * CD:3 * CD], in_=co[:, 2 * CD:3 * CD])

            # v = squash(y/z) with y = s_sum, z = Z (it0: z == R), computed
            # without dividing:  v = y*y^2 / ((z^2+y^2)(|y| + eps*z))
            # final iteration runs per batch-half so the output DMA starts early
            halves = [(0, 2 * CD)] if it < NITER - 1 else [(0, CD), (CD, 2 * CD)]
            zsq = None
            for lo, hi in halves:
                hw_ = hi - lo
                nh = hw_ // CD
                y = red[:, lo:hi]
                sq = work.tile([128, hw_], f32, tag=f"sq{lo}", name=f"sq{lo}")
                nc.vector.tensor_mul(sq, y, y)
                ay = work.tile([128, hw_], bf16, tag=f"ax{lo}", name=f"ax{lo}")
                nc.vector.tensor_scalar(
                    ay.bitcast(mybir.dt.uint16), y.bitcast(mybir.dt.uint16),
                    0x7FFF, None, ALU.bitwise_and,
                )
                if it > 0 and zsq is None:
                    zsq = work.tile([128, CD], f32, tag="zsq")
                    nc.vector.tensor_mul(zsq, red[:, 2 * CD:3 * CD],
                                         red[:, 2 * CD:3 * CD])
                d2 = work.tile([128, hw_], f32, tag=f"d2{lo}", name=f"d2{lo}")
                den = work.tile([128, hw_], f32, tag=f"dn{lo}", name=f"dn{lo}")
                if it == 0:
                    nc.vector.tensor_scalar_add(d2, sq, float(R) * R)
                    aze = work.tile([128, hw_], f32, tag=f"az{lo}", name=f"az{lo}")
                    nc.vector.tensor_scalar_add(aze, ay, EPS * R)
                    nc.vector.tensor_mul(den, d2, aze)
                else:
                    z2 = bass.AP(tensor=zsq.tensor, offset=zsq.offset,
                                 ap=[zsq.ap[0], [0, nh], [1, CD]])
                    nc.vector.tensor_add(
                        d2.rearrange("p (h f) -> p h f", f=CD),
                        sq.rearrange("p (h f) -> p h f", f=CD),
                        z2,
                    )
                    rz_sl = red[:, 2 * CD:3 * CD]
                    zb = bass.AP(tensor=rz_sl.tensor, offset=rz_sl.offset,
                                 ap=[rz_sl.ap[0], [0, nh], [1, CD]])
                    aze = work.tile([128, hw_], f32, tag=f"az{lo}", name=f"az{lo}")
                    nc.vector.scalar_tensor_tensor(
                        out=aze.rearrange("p (h f) -> p h f", f=CD),
                        in0=zb, scalar=EPS,
                        in1=ay.rearrange("p (h f) -> p h f", f=CD),
                        op0=ALU.mult, op1=ALU.add,
                    )
                    nc.vector.tensor_mul(den, d2, aze)
                rr = work.tile([128, hw_], f32, tag=f"rr{lo}", name=f"rr{lo}")
                nc.vector.reciprocal_approx_fast(out=rr, in_=den)
                n1 = work.tile([128, hw_], f32, tag=f"n1{lo}", name=f"n1{lo}")
                nc.vector.tensor_mul(n1, y, sq)
                if it < NITER - 1:
                    # v feeds only the bf16 b-update matmuls
                    nc.vector.tensor_mul(sb_vb, n1, rr)
                else:
                    nc.vector.tensor_mul(sb_vf[:, lo:hi], n1, rr)
                    bh = lo // CD
                    nc.sync.dma_start(
                        out=out_d[bh * 128:(bh + 1) * 128, :],
                        in_=sb_vf[:, lo:hi],
                    )

            if it < NITER - 1:
                # b += sum_i (Wd/B) * (u x v):  M_k = sum_b u v  (batch on parts)
                for g in range(NG):
                    bu = ps_bu.tile([16, NG * CD], f32, tag="bu")
                    gg = gpool.tile([128, NG * CD], bf16, tag="gk")
                    for j in range(NG):
                        k = g * NG + j
                        mp = ps_m.tile([128, CD], f32, tag="mp")
                        for bh in range(2):
                            nc.tensor.matmul(
                                mp,
                                sb_uBb[:, bh * RL * I + k * 128: bh * RL * I + (k + 1) * 128],
                                sb_vb[:, bh * CD:(bh + 1) * CD],
                                start=(bh == 0), stop=(bh == 1),
                            )
                        wd_b = bass.AP(
                            tensor=sb_Wd.tensor, offset=sb_Wd.offset + k * C,
                            ap=[sb_Wd.ap[0], [1, C], [0, D]],
                        )
                        nc.vector.tensor_mul(
                            gg[:, j * CD:(j + 1) * CD].rearrange(
                                "p (c d) -> p c d", d=D),
                            mp.rearrange("p (c d) -> p c d", d=D),
                            wd_b,
                        )
                    # one i-reduce matmul per group (n=480)
                    nc.tensor.matmul(bu, sb_obdb, gg, start=True, stop=True)
                    nc.vector.tensor_add(
                        sb_b[:, g * NG * CD:(g + 1) * NG * CD],
                        sb_b[:, g * NG * CD:(g + 1) * NG * CD],
                        bu,
                    )

    nc.compile()
    return nc


def _host_inputs(u, W):
    """Pure-permutation host prep: per-core (r,i)-major and b-major layouts."""
    u = np.ascontiguousarray(u, dtype=np.float32)
    W = np.ascontiguousarray(W, dtype=np.float32)
    obd = np.zeros((128, 16), dtype=np.float32)
    for p in range(128):
        obd[p, p // 8] = 1.0
    ex2 = obd.T.copy()  # (16, 128)
    in_maps = []
    for cidx in range(NCORES):
        rs = cidx * RL
        usl = u[:, rs:rs + RL, :].reshape(B, RL * I)           # (256, 1152)
        uTd = (usl.T.reshape(NCHUNK, 128, B)
               .transpose(1, 0, 2).reshape(128, NCHUNK * B))
        uBd = (usl.reshape(2, 128, RL * I)
               .transpose(1, 0, 2).reshape(128, 2 * RL * I))
        wsl = W[rs:rs + RL].transpose(0, 3, 1, 2).reshape(RL * I, CD)
        Wtd = (wsl.reshape(NCHUNK, 128, CD)
               .transpose(1, 0, 2).reshape(128, NCHUNK * CD))
        data = np.zeros((128, DW), dtype=np.float32)
        data[:, O_UT:O_UT + NCHUNK * B] = uTd
        data[:, O_UB:O_UB + 2 * RL * I] = uBd
        data[:, O_WT:O_WT + NCHUNK * CD] = Wtd
        data[:, O_OBD:O_OBD + 16] = obd
        data[:16, O_EX2:O_EX2 + 128] = ex2
        in_maps.append({"data": data})
    return in_maps


def _install_profile_hook():
    """Recreate the missing antenv.axon_hooks NTFF-profile hook (dev only)."""
    import contextlib
    import ctypes
    import sys
    import types

    try:
        from antenv.axon_hooks import get_axon_ntff_profile_hook  # noqa: F401
        return
    except ImportError:
        pass

    mod = types.ModuleType("antenv.axon_hooks")
    holder = {}
    mod.set_axon_ntff_profile_hook = lambda h: holder.__setitem__("h", h)
    mod.get_axon_ntff_profile_hook = lambda: holder.get("h")
    import antenv

    sys.modules["antenv.axon_hooks"] = mod
    antenv.axon_hooks = mod

    so_path = "/opt/axon/libaxon_pjrt.so"
    lib = ctypes.CDLL(so_path)
    if not hasattr(lib, "axon_start_nrt_profile"):
        return
    lib.axon_start_nrt_profile.argtypes = [
        ctypes.POINTER(ctypes.c_int64),
        ctypes.c_size_t,
    ]
    lib.axon_start_nrt_profile.restype = ctypes.c_int64
    lib.axon_stop_nrt_profile.argtypes = [ctypes.c_char_p]
    lib.axon_stop_nrt_profile.restype = ctypes.c_int64

    @contextlib.contextmanager
    def _hook(output_dir, device_ids):
        import jax

        jax.devices()
        if device_ids:
            ids = (ctypes.c_int64 * len(device_ids))(*device_ids)
            rc = lib.axon_start_nrt_profile(ids, len(device_ids))
        else:
            rc = lib.axon_start_nrt_profile(None, 0)
        if rc != 0:
            raise RuntimeError(f"axon_start_nrt_profile rc={rc}")
        try:
            yield
        finally:
            n = lib.axon_stop_nrt_profile(str(output_dir).encode())
            print(f"profile: {n} file(s) written to {output_dir}")

    mod.set_axon_ntff_profile_hook(_hook)

    # Avoid the bucket upload inside the trace post-processing.
    import concourse.bass_utils as bu

    bu.upload_artifacts = lambda tmpdir: f"local:{tmpdir}"


def kernel(u, W):
    from concourse.bass_utils import run_bass_kernel_spmd

    if os.environ.get("KERNEL_TRACE", "0") == "1":
        _install_profile_hook()
    if "nc" not in _CACHE:
        _CACHE["nc"] = _build_program()
    nc = _CACHE["nc"]
    in_maps = _host_inputs(u, W)
    trace = os.environ.get("KERNEL_TRACE", "0") == "1"
    res = run_bass_kernel_spmd(
        nc, in_maps, core_ids=list(range(NCORES)), trace=trace
    )
    _CACHE["last_result"] = res
    return np.asarray(res.results[0]["out"]).reshape(B, C, D)
